# revision 1
# baseline (speedup 1.0000x reference)
"""EGAT (edge-featured GAT) kernel for 8 Trainium2 NeuronCores.

Edge-parallel sharding: edges are sorted by dst and split into 8 contiguous
shards at dst boundaries, so each core owns a disjoint dst range end-to-end
(softmax stats + aggregation are fully local -> no collectives).

Per core the edges are packed into W windows of 2048 edge slots (16 tiles of
128).  A window spans at most 128 distinct dst rows.  The host gathers the
src/dst feature rows per edge slot (edge-aligned layout, transposed so the
feature dim sits on partitions); the device projects them tile-by-tile on
the TensorEngine, accumulating f_ni + f_nj + r*wsum + b_e in one PSUM tile
(r*wsum and b_e ride along via host-crafted virtual feature rows x_row and
y_row with x_row @ W_nj = colsum(W_fij), y_row @ W_nj = b_e).  Attention
weights (leaky_relu -> attn dot -> exp, no max-subtraction needed: logits
are O(1)) are computed on DVE/ACT, and a one-hot scatter matmul accumulates
both the weighted messages and the softmax denominators into one PSUM tile
per window.  The epilogue normalizes, averages heads, adds mean(b_ns),
applies relu, and DMAs the window's 128 output rows.
"""

import sys

sys.path.insert(0, "/opt/trn_rl_repo")

import numpy as np
import ml_dtypes

BF16 = ml_dtypes.bfloat16

# ---- problem constants (hardcoded per the task contract) -------------------
N_SRC = 50000
N_DST = 50000
E = 800000
IN_NODE = 128
IN_EDGE = 16
OUT_NODE = 64
OUT_EDGE = 16
H = 4
SLOPE = 0.01

N_CORES = 8


def default_cfg():
    return dict(
        n_dst=N_DST,
        t_half=8,             # tiles per half-window (8 -> 1024 slots/half)
        span=128,             # max dst rows per window
    )


# ===========================================================================
# Host-side packing
# ===========================================================================

def prep(nfeats, dst_feats, reward, src, dst, W_ns, b_ns, W_ni, W_nj, W_fij,
         attn, b_e, cfg=None):
    """Sort/shard/pack everything. Returns (meta, in_maps)."""
    cfg = cfg or default_cfg()
    n_dst = cfg["n_dst"]
    t_half = cfg["t_half"]
    span = cfg["span"]
    slots = 2 * t_half * 128    # slots per window
    t_w = 2 * t_half

    e_tot = src.shape[0]

    nfeats = np.asarray(nfeats, np.float32)
    dst_feats = np.asarray(dst_feats, np.float32)
    reward = np.asarray(reward, np.float32)
    src = np.asarray(src, np.int64)
    dst = np.asarray(dst, np.int64)
    W_ns = np.asarray(W_ns, np.float32)
    b_ns = np.asarray(b_ns, np.float32)
    W_ni = np.asarray(W_ni, np.float32)
    W_nj = np.asarray(W_nj, np.float32)
    W_fij = np.asarray(W_fij, np.float32)
    attn = np.asarray(attn, np.float32)
    b_e = np.asarray(b_e, np.float32)

    # ---- sort by dst and shard at dst boundaries --------------------------
    order = np.argsort(dst, kind="stable")
    d_s = dst[order]
    s_s = src[order]
    r_s = reward[order]

    cut = [0]
    for c in range(1, N_CORES):
        t = (e_tot * c) // N_CORES
        while t < e_tot and t > 0 and d_s[t] == d_s[t - 1]:
            t += 1
        cut.append(t)
    cut.append(e_tot)

    # ---- greedy window packing per core -----------------------------------
    per_core = []
    for c in range(N_CORES):
        e0, e1 = cut[c], cut[c + 1]
        d = d_s[e0:e1]
        wins = []  # (base, n_edges) over local positions (contiguous runs)
        if e1 > e0:
            uniq, starts = np.unique(d, return_index=True)
            ends = np.append(starts[1:], len(d))
            base = None
            w_start = 0
            w_count = 0
            for gi in range(len(uniq)):
                dd = int(uniq[gi])
                glen = int(ends[gi] - starts[gi])
                if (base is None or dd - base > span - 1
                        or w_count + glen > slots):
                    if base is not None:
                        wins.append((base, w_start, w_count))
                    base = dd
                    w_start = int(starts[gi])
                    w_count = 0
                w_count += glen
            wins.append((base, w_start, w_count))
        per_core.append((e0, e1, wins))

    W = max(1, max(len(pc[2]) for pc in per_core))

    # virtual feature rows: x_row @ W_nj == colsum(W_fij); y_row @ W_nj == b_e
    wsum = W_fij.sum(axis=0)
    x_row = np.linalg.lstsq(W_nj.T.astype(np.float64), wsum.astype(np.float64),
                            rcond=None)[0].astype(np.float32)
    y_row = np.linalg.lstsq(W_nj.T.astype(np.float64), b_e.astype(np.float64),
                            rcond=None)[0].astype(np.float32)

    mf_all = []     # [128, W, t_w, 128] bf16 one-hot per slot
    zfe_all = []    # [128, W*2*slots] bf16 (src feats | dst feats per window)
    asm = []        # per core (slot_rows, global_rows)

    for c in range(N_CORES):
        e0, e1, wins = per_core[c]
        d = d_s[e0:e1]
        s = s_s[e0:e1]
        r = r_s[e0:e1]

        drel = np.full((W, slots), -1.0, np.float32)
        nfe = np.zeros((W * slots, IN_NODE), np.float32)
        dfe = np.zeros((W * slots, IN_NODE), np.float32)
        rows_slot = []
        rows_glob = []
        for w, (base, ws, wc) in enumerate(wins):
            sl = slice(ws, ws + wc)
            drel[w, :wc] = (d[sl] - base).astype(np.float32)
            nfe[w * slots:w * slots + wc] = nfeats[s[sl]]
            dfe[w * slots:w * slots + wc] = (dst_feats[d[sl]]
                                             + r[sl, None] * x_row[None, :]
                                             + y_row[None, :])
            uds = np.unique(d[sl])
            rows_slot.append(w * 128 + (uds - base))
            rows_glob.append(uds)

        # one-hot per slot, layout [128 p, W, t, 128 dcol]
        ohm = (drel.reshape(W, t_w, 128)[:, :, :, None]
               == np.arange(128, dtype=np.float32)).astype(BF16)
        ohm = np.ascontiguousarray(ohm.transpose(2, 0, 1, 3))

        zfe = np.empty((IN_NODE, W, 2 * slots), np.float32)
        zfe[:, :, :slots] = nfe.T.reshape(IN_NODE, W, slots)
        zfe[:, :, slots:] = dfe.T.reshape(IN_NODE, W, slots)
        mf_all.append(ohm)
        zfe_all.append(np.ascontiguousarray(
            zfe.reshape(IN_NODE, W * 2 * slots).astype(BF16)))
        asm.append((np.concatenate(rows_slot) if rows_slot else
                    np.zeros(0, np.int64),
                    np.concatenate(rows_glob) if rows_glob else
                    np.zeros(0, np.int64)))

    # ---- shared constants -------------------------------------------------
    wcat = np.concatenate([W_ni, W_ns * 0.25], axis=1).astype(BF16)  # [128,320]
    wnj = W_nj.astype(BF16)                                          # [128,64]
    attn_rep = np.broadcast_to(attn.reshape(-1).astype(np.float32),
                               (128, H * OUT_EDGE)).astype(BF16).copy()
    bmean = np.broadcast_to(b_ns.reshape(H, OUT_NODE).mean(axis=0),
                            (128, OUT_NODE)).astype(np.float32).copy()

    in_maps = []
    for c in range(N_CORES):
        in_maps.append(dict(
            zfe=zfe_all[c], ohm=mf_all[c],
            wcat=wcat, wnj=wnj, attn_rep=attn_rep, bmean=bmean,
        ))

    meta = dict(W=W, asm=asm, cfg=cfg)
    return meta, in_maps


# ===========================================================================
# Device program
# ===========================================================================

def build_program(W, cfg):
    import concourse.bacc as bacc
    import concourse.tile as tile
    import concourse.mybir as mybir
    from contextlib import ExitStack

    dt = mybir.dt
    AF = mybir.ActivationFunctionType
    OP = mybir.AluOpType

    t_half = cfg["t_half"]
    t_w = 2 * t_half
    slots = t_w * 128
    FE = H * OUT_EDGE          # 64
    NPAY = H * OUT_NODE        # 256
    NPROJ = FE + NPAY          # 320

    nc = bacc.Bacc(None, target_bir_lowering=False)

    ZFE = nc.declare_dram_parameter("zfe", [IN_NODE, W * 2 * slots],
                                    dt.bfloat16, isOutput=False)
    OHM = nc.declare_dram_parameter("ohm", [128, W, 2 * t_half, 128],
                                    dt.bfloat16, isOutput=False)
    WCAT = nc.declare_dram_parameter("wcat", [IN_NODE, NPROJ], dt.bfloat16,
                                     isOutput=False)
    WNJ = nc.declare_dram_parameter("wnj", [IN_NODE, FE], dt.bfloat16,
                                    isOutput=False)
    ATTN = nc.declare_dram_parameter("attn_rep", [128, FE], dt.bfloat16,
                                     isOutput=False)
    BMEAN = nc.declare_dram_parameter("bmean", [128, OUT_NODE], dt.float32,
                                      isOutput=False)
    OUT = nc.declare_dram_parameter("out", [W * 128, OUT_NODE], dt.float32,
                                    isOutput=True)

    with tile.TileContext(nc) as tc, ExitStack() as ctx:
        cpool = ctx.enter_context(tc.tile_pool(name="consts", bufs=1))
        wcat_s = cpool.tile([128, NPROJ], dt.bfloat16)
        nc.sync.dma_start(wcat_s[:], WCAT[:])
        wnj_s = cpool.tile([128, FE], dt.bfloat16)
        nc.sync.dma_start(wnj_s[:], WNJ[:])
        out_acc = cpool.tile([128, W, OUT_NODE], dt.float32)
        attn_s = cpool.tile([128, FE], dt.bfloat16)
        nc.sync.dma_start(attn_s[:], ATTN[:])
        bmean_s = cpool.tile([128, OUT_NODE], dt.float32)
        nc.sync.dma_start(bmean_s[:], BMEAN[:])

        with tc.tile_pool(name="feat", bufs=3) as fpool, \
             tc.tile_pool(name="meta", bufs=3) as mpool, \
             tc.tile_pool(name="work", bufs=3) as wpool, \
             tc.tile_pool(name="stgp", bufs=3) as ppool, \
             tc.tile_pool(name="rhsp", bufs=3) as rpool, \
             tc.tile_pool(name="ep", bufs=2) as epool, \
             tc.tile_pool(name="psPr", bufs=2, space="PSUM") as psPr, \
             tc.tile_pool(name="psP", bufs=2, space="PSUM") as psP:
            for w in range(W):
                zfe = fpool.tile([128, 2 * slots], dt.bfloat16, tag="zfe")
                eng = nc.sync if w % 2 == 0 else nc.gpsimd
                eng.dma_start(zfe[:],
                              ZFE[:, w * 2 * slots:(w + 1) * 2 * slots])
                nfe = zfe[:, 0:slots]
                dfe = zfe[:, slots:2 * slots]
                ohw = mpool.tile([128, 2 * t_half, 128], dt.bfloat16, tag="ohw")
                (nc.gpsimd if w % 2 == 0 else nc.sync).dma_start(
                    ohw[:], OHM[:, w, :, :])

                P = psP.tile([128, NPAY + H], dt.float32, tag="P")
                rhs = rpool.tile([128, t_w, NPAY + H], dt.bfloat16, tag="rhs")
                stg = ppool.tile([128, t_w, NPROJ], dt.bfloat16, tag="stg")

                for hf in range(2):
                    ts0 = hf * t_half
                    oh = ohw[:, ts0:ts0 + t_half, :]

                    lr = wpool.tile([128, t_half, FE], dt.bfloat16, tag="lr")
                    # projection: 2 tiles per PSUM chunk (2 banks)
                    for half_t in range(t_half // 2):
                        pr = psPr.tile([128, 2, 512], dt.float32, tag="pr")
                        for k in range(2):
                            t = ts0 + half_t * 2 + k
                            c0 = t * 128
                            nc.tensor.matmul(
                                pr[:, k, 0:NPROJ], lhsT=nfe[:, c0:c0 + 128],
                                rhs=wcat_s[:], start=True, stop=True,
                                skip_group_check=True)
                            nc.tensor.matmul(
                                pr[:, k, 0:FE], lhsT=dfe[:, c0:c0 + 128],
                                rhs=wnj_s[:], start=False, stop=True,
                                skip_group_check=True)
                        tl = ts0 + half_t * 2
                        # stage the whole projection out of PSUM (bf16)
                        nc.scalar.copy(stg[:, tl:tl + 2, :],
                                       pr[:, :, 0:NPROJ])

                    # leaky relu, batched over the half-window
                    fo = stg[:, ts0:ts0 + t_half, 0:FE]
                    nc.vector.scalar_tensor_tensor(
                        out=lr[:], in0=fo, scalar=SLOPE, in1=fo,
                        op0=OP.mult, op1=OP.max)
                    ea = wpool.tile([128, t_half, FE], dt.bfloat16, tag="ea")
                    nc.vector.tensor_tensor(
                        out=ea[:], in0=lr[:],
                        in1=attn_s[:].unsqueeze(1).broadcast_to(
                            [128, t_half, FE]),
                        op=OP.mult)
                    eat = wpool.tile([128, t_half, H], dt.float32, tag="eat")
                    nc.vector.tensor_reduce(
                        eat[:], ea[:].rearrange("p t (h f) -> p t h f",
                                                f=OUT_EDGE),
                        axis=mybir.AxisListType.X, op=OP.add)
                    nc.scalar.activation(rhs[:, ts0:ts0 + t_half, NPAY:],
                                         eat[:], AF.Exp)
                    nc.vector.tensor_tensor(
                        out=rhs[:, ts0:ts0 + t_half, 0:NPAY]
                        .rearrange("p t (h f) -> p t h f", f=OUT_NODE),
                        in0=stg[:, ts0:ts0 + t_half, FE:NPROJ]
                        .rearrange("p t (h f) -> p t h f", f=OUT_NODE),
                        in1=rhs[:, ts0:ts0 + t_half, NPAY:].unsqueeze(3)
                        .broadcast_to([128, t_half, H, OUT_NODE]),
                        op=OP.mult)
                    for t in range(t_half):
                        tg = ts0 + t
                        nc.tensor.matmul(P[:], lhsT=oh[:, t, :],
                                         rhs=rhs[:, tg, :],
                                         start=(tg == 0), stop=(tg == t_w - 1),
                                         skip_group_check=True)

                # ---- epilogue -------------------------------------------
                sg = epool.tile([128, H], dt.float32, tag="sg")
                nc.vector.tensor_scalar(out=sg[:], in0=P[:, NPAY:],
                                        scalar1=1e-30, scalar2=None,
                                        op0=OP.max)
                si = epool.tile([128, H], dt.float32, tag="si")
                nc.vector.reciprocal(si[:], sg[:])
                tmp = epool.tile([128, OUT_NODE, H], dt.float32, tag="tmp")
                nc.vector.tensor_tensor(
                    out=tmp[:].transpose([0, 2, 1]),
                    in0=P[:, 0:NPAY].rearrange("p (h f) -> p h f", f=OUT_NODE),
                    in1=si[:].unsqueeze(2).broadcast_to([128, H, OUT_NODE]),
                    op=OP.mult)
                acc = epool.tile([128, OUT_NODE], dt.float32, tag="acc")
                nc.vector.tensor_reduce(acc[:], tmp[:],
                                        axis=mybir.AxisListType.X, op=OP.add)
                m01 = epool.tile([128, 1], dt.float32, tag="m01")
                nc.vector.tensor_scalar(out=m01[:], in0=P[:, NPAY:NPAY + 1],
                                        scalar1=0.0, scalar2=None, op0=OP.is_gt)
                acc2 = epool.tile([128, OUT_NODE], dt.float32, tag="acc2")
                nc.vector.tensor_add(acc2[:], acc[:], bmean_s[:])
                nc.vector.tensor_scalar(out=out_acc[:, w, :], in0=acc2[:],
                                        scalar1=0.0, scalar2=m01[:],
                                        op0=OP.max, op1=OP.mult)

            nc.sync.dma_start(OUT[:].rearrange("(w p) c -> p w c", p=128),
                              out_acc[:])

    if not nc.is_finalized():
        nc.finalize()
    return nc


# ===========================================================================
# numpy emulation of the device program (for validation/debug)
# ===========================================================================

def emulate_core(in_map, W, cfg):
    t_half = cfg["t_half"]
    slots = 2 * t_half * 128
    FE = H * OUT_EDGE
    NPAY = H * OUT_NODE

    f32 = np.float32
    wcat = in_map["wcat"].astype(f32)
    wnj = in_map["wnj"].astype(f32)
    attn_rep = in_map["attn_rep"][0].astype(f32)
    bmean = in_map["bmean"][0]

    out = np.zeros((W * 128, OUT_NODE), f32)
    for w in range(W):
        zfe = in_map["zfe"][:, w * 2 * slots:(w + 1) * 2 * slots].astype(f32)
        nfe = zfe[:, 0:slots].T
        dfe = zfe[:, slots:].T
        proj = nfe @ wcat                       # [slots, 320] (psum f32)
        proj[:, 0:FE] += dfe @ wnj
        pay = proj[:, FE:].astype(BF16).astype(f32)
        fout = proj[:, 0:FE].astype(BF16).astype(f32)
        lr = np.maximum(fout, SLOPE * fout).astype(BF16).astype(f32)
        eat = ((lr * attn_rep[None, :]).astype(BF16).astype(f32)
               .reshape(-1, H, OUT_EDGE).sum(axis=2))
        wgt = np.exp(eat).astype(BF16).astype(f32)          # [slots, H]
        oh = (in_map["ohm"][:, w].astype(f32).transpose(1, 0, 2)
              .reshape(slots, 128))
        rhs = np.concatenate(
            [(pay.reshape(-1, H, OUT_NODE)
              * wgt[:, :, None]).reshape(-1, NPAY).astype(BF16).astype(f32),
             wgt], axis=1)
        P = oh.T @ rhs                                       # [128, 260]
        s = np.maximum(P[:, NPAY:], 1e-30)
        acc = (P[:, 0:NPAY].reshape(128, H, OUT_NODE) /
               s[:, :, None]).sum(axis=1)
        m01 = (P[:, NPAY:NPAY + 1] > 0).astype(f32)
        out[w * 128:(w + 1) * 128] = np.maximum(acc + bmean[None, :], 0) * m01
    return out


def assemble(meta, results):
    n_dst = meta["cfg"]["n_dst"]
    out = np.zeros((n_dst, OUT_NODE), np.float32)
    for c in range(N_CORES):
        slots_rows, glob_rows = meta["asm"][c]
        if len(glob_rows):
            out[glob_rows] = results[c]["out"][slots_rows]
    return out


# ===========================================================================
# entry point
# ===========================================================================

_CACHE = {}
LAST_EXEC_NS = None
LAST_RESULT = None


def kernel(nfeats, dst_feats, reward, src, dst,
           W_ns, b_ns, W_ni, W_nj, W_fij, attn, b_e):
    global LAST_EXEC_NS, LAST_RESULT
    import os
    from concourse.bass_utils import run_bass_kernel_spmd

    meta, in_maps = prep(nfeats, dst_feats, reward, src, dst,
                         W_ns, b_ns, W_ni, W_nj, W_fij, attn, b_e)
    key = meta["W"]
    if key not in _CACHE:
        _CACHE[key] = build_program(meta["W"], meta["cfg"])
    nc = _CACHE[key]
    kwargs = {}
    if os.environ.get("EGAT_TRACE"):
        kwargs = dict(trace=True)
    try:
        res = run_bass_kernel_spmd(nc, in_maps, list(range(N_CORES)), **kwargs)
    except ModuleNotFoundError:
        # NTFF profile hook unavailable in this environment
        res = run_bass_kernel_spmd(nc, in_maps, list(range(N_CORES)))
    LAST_EXEC_NS = res.exec_time_ns
    LAST_RESULT = res
    return assemble(meta, res.results)


def estimate_ns(W=None, cfg=None):
    """Cost-model (no_exec CoreSim) estimate of the per-core kernel time."""
    from concourse.bass_interp import CoreSim
    cfg = cfg or default_cfg()
    if W is None:
        W = sorted(_CACHE)[0] if _CACHE else 50
    nc = _CACHE.get(W) or build_program(W, cfg)
    sim = CoreSim(nc, no_exec=True)
    sim.simulate()
    return int(sim.time)



# revision 27
# speedup vs baseline: 2.1204x; 2.1204x over previous
"""EGAT (edge-featured GAT) kernel for 8 Trainium2 NeuronCores.

Edge-parallel sharding: edges sorted by dst, split into 8 contiguous shards at
dst boundaries -> each core owns a disjoint dst range (softmax stats and
aggregation fully local, no collectives).

Per core, edges pack into W windows of 2048 slots (16 tiles x 128); a window
spans <=128 distinct dst rows.  Host gathers per-slot src/dst feature rows
(feature-major) plus a one-hot dst matrix per slot.  Device pipeline per
window, engine-balanced (~4.5us/window each on PE/ACT/DVE/GPSIMD):

  PE:    per-tile projections  attn: A=ni+nj (PSUM [128,8,64] per half),
         payload C=ns (PSUM [128,4,256] per quarter, (f,h)-permuted cols),
         one-hot scatter accumulating P[dst,260] over the window.
  ACT:   leaky-relu A->lr (Lrelu activation, PSUM->SBUF bf16), exp(eat)->wgt,
         and staging of half the payload quarters PSUM->SBUF bf16.
  GPSIMD: ea = lr*attn, eat = pairwise head-dot sums, epilogue tail.
  DVE:   payload x wgt multiplies (staged quarters at 2x thanks to the
         h-innermost (f,h) layout; unstaged quarters fused from PSUM),
         softmax normalization epilogue.

The reward/b_e edge terms ride along in the dst features via host-crafted
virtual rows (x_row @ W_nj == colsum(W_fij), y_row @ W_nj == b_e).  The /H
head-mean is folded into the payload weights; b_ns's mean is added in the
epilogue with a (s>0) mask.
"""

import sys

sys.path.insert(0, "/opt/trn_rl_repo")

import os
import numpy as np
import ml_dtypes

BF16 = ml_dtypes.bfloat16
FP8 = ml_dtypes.float8_e4m3

# ---- problem constants (hardcoded per the task contract) -------------------
N_SRC = 50000
N_DST = 50000
E = 800000
IN_NODE = 128
IN_EDGE = 16
OUT_NODE = 64
OUT_EDGE = 16
H = 4
SLOPE = 0.01

N_CORES = 8

FE = H * OUT_EDGE            # 64 attention cols (h,f)
NPAY = H * OUT_NODE          # 256 payload cols, stored (f,h) h-innermost
NR = NPAY + H                # 260 rhs cols (payload + per-head weights)

DFE_FP8 = not os.environ.get("EGAT_DFE_BF16")
# debug switches for engine bisection
DBG_LR_DVE = bool(os.environ.get("EGAT_LR_DVE"))
DBG_EA_DVE = bool(os.environ.get("EGAT_EA_DVE"))
DBG_EAT_DVE = bool(os.environ.get("EGAT_EAT_DVE"))


def default_cfg():
    return dict(
        n_dst=N_DST,
        t_half=8,             # tiles per half-window
        span=128,             # max dst rows per window
    )


# ===========================================================================
# Host-side packing
# ===========================================================================

def prep(nfeats, dst_feats, reward, src, dst, W_ns, b_ns, W_ni, W_nj, W_fij,
         attn, b_e, cfg=None):
    """Sort/shard/pack everything. Returns (meta, in_maps)."""
    cfg = cfg or default_cfg()
    t_half = cfg["t_half"]
    span = cfg["span"]
    slots = 2 * t_half * 128
    t_w = 2 * t_half

    e_tot = src.shape[0]

    nfeats = np.asarray(nfeats, np.float32)
    dst_feats = np.asarray(dst_feats, np.float32)
    reward = np.asarray(reward, np.float32)
    src = np.asarray(src, np.int64)
    dst = np.asarray(dst, np.int64)
    W_ns = np.asarray(W_ns, np.float32)
    b_ns = np.asarray(b_ns, np.float32)
    W_ni = np.asarray(W_ni, np.float32)
    W_nj = np.asarray(W_nj, np.float32)
    W_fij = np.asarray(W_fij, np.float32)
    attn = np.asarray(attn, np.float32)
    b_e = np.asarray(b_e, np.float32)

    # ---- sort by dst and shard at dst boundaries --------------------------
    order = np.argsort(dst, kind="stable")
    d_s = dst[order]
    s_s = src[order]
    r_s = reward[order]

    cut = [0]
    for c in range(1, N_CORES):
        t = (e_tot * c) // N_CORES
        while t < e_tot and t > 0 and d_s[t] == d_s[t - 1]:
            t += 1
        cut.append(t)
    cut.append(e_tot)

    # ---- greedy window packing per core -----------------------------------
    per_core = []
    for c in range(N_CORES):
        e0, e1 = cut[c], cut[c + 1]
        d = d_s[e0:e1]
        wins = []
        if e1 > e0:
            uniq, starts = np.unique(d, return_index=True)
            ends = np.append(starts[1:], len(d))
            base = None
            w_start = 0
            w_count = 0
            for gi in range(len(uniq)):
                dd = int(uniq[gi])
                glen = int(ends[gi] - starts[gi])
                if (base is None or dd - base > span - 1
                        or w_count + glen > slots):
                    if base is not None:
                        wins.append((base, w_start, w_count))
                    base = dd
                    w_start = int(starts[gi])
                    w_count = 0
                w_count += glen
            wins.append((base, w_start, w_count))
        per_core.append((e0, e1, wins))

    W = max(1, max(len(pc[2]) for pc in per_core))

    # virtual feature rows: x_row @ W_nj == colsum(W_fij); y_row @ W_nj == b_e
    wsum = W_fij.sum(axis=0)
    x_row = np.linalg.lstsq(W_nj.T.astype(np.float64), wsum.astype(np.float64),
                            rcond=None)[0].astype(np.float32)
    y_row = np.linalg.lstsq(W_nj.T.astype(np.float64), b_e.astype(np.float64),
                            rcond=None)[0].astype(np.float32)

    dfe_t = FP8 if DFE_FP8 else BF16

    zfe_all = []
    dfe_all = []
    ohm_all = []
    asm = []

    for c in range(N_CORES):
        e0, e1, wins = per_core[c]
        d = d_s[e0:e1]
        s = s_s[e0:e1]
        r = r_s[e0:e1]

        drel = np.full((W, slots), -1.0, np.float32)
        nfe = np.zeros((W * slots, IN_NODE), np.float32)
        dfe = np.zeros((W * slots, IN_NODE), np.float32)
        rows_slot = []
        rows_glob = []
        for w, (base, ws, wc) in enumerate(wins):
            sl = slice(ws, ws + wc)
            drel[w, :wc] = (d[sl] - base).astype(np.float32)
            nfe[w * slots:w * slots + wc] = nfeats[s[sl]]
            dfe[w * slots:w * slots + wc] = (dst_feats[d[sl]]
                                             + r[sl, None] * x_row[None, :]
                                             + y_row[None, :])
            uds = np.unique(d[sl])
            rows_slot.append(w * 128 + (uds - base))
            rows_glob.append(uds)

        # one-hot per slot, layout [128 p, W, t, 128 dcol]; 0/1 exact in fp8
        ohm = (drel.reshape(W, t_w, 128)[:, :, :, None]
               == np.arange(128, dtype=np.float32)).astype(FP8)
        ohm = np.ascontiguousarray(ohm.transpose(2, 0, 1, 3))

        zfe_all.append(np.ascontiguousarray(
            nfe.T.reshape(IN_NODE, W * slots).astype(BF16)))
        # fp8 attention features: per tile, (nfe | dfe) k-tile pair for the
        # DoubleRow fused projection
        aq = np.stack([nfe.T.reshape(IN_NODE, W * t_w, 128).astype(FP8),
                       dfe.T.reshape(IN_NODE, W * t_w, 128).astype(FP8)],
                      axis=2)
        dfe_all.append(np.ascontiguousarray(aq))
        ohm_all.append(ohm)
        asm.append((np.concatenate(rows_slot) if rows_slot else
                    np.zeros(0, np.int64),
                    np.concatenate(rows_glob) if rows_glob else
                    np.zeros(0, np.int64)))

    # ---- shared constants -------------------------------------------------
    wq = np.ascontiguousarray(np.stack(
        [W_ni.astype(FP8), W_nj.astype(FP8)], axis=1))         # [128, 2, 64]
    # payload weights, (f,h) h-innermost, /H mean folded in
    wpay = np.ascontiguousarray(
        (W_ns * (1.0 / H)).reshape(IN_NODE, H, OUT_NODE)
        .transpose(0, 2, 1).reshape(IN_NODE, NPAY)).astype(BF16)
    attn_rep = np.broadcast_to(attn.reshape(-1).astype(np.float32),
                               (128, FE)).astype(BF16).copy()
    bmean = np.broadcast_to(b_ns.reshape(H, OUT_NODE).mean(axis=0),
                            (128, OUT_NODE)).astype(np.float32).copy()

    in_maps = []
    for c in range(N_CORES):
        in_maps.append(dict(
            zfe=zfe_all[c], aq=dfe_all[c], ohm=ohm_all[c],
            wq=wq, wpay=wpay, attn_rep=attn_rep, bmean=bmean,
        ))

    meta = dict(W=W, asm=asm, cfg=cfg)
    return meta, in_maps


# ===========================================================================
# Device program
# ===========================================================================

def build_program(W, cfg):
    import concourse.bacc as bacc
    import concourse.tile as tile
    import concourse.mybir as mybir
    from contextlib import ExitStack

    dt = mybir.dt
    AF = mybir.ActivationFunctionType
    OP = mybir.AluOpType

    t_half = cfg["t_half"]
    t_w = 2 * t_half
    slots = t_w * 128
    dfe_dt = dt.float8e4 if DFE_FP8 else dt.bfloat16

    nc = bacc.Bacc(None, target_bir_lowering=False)

    ZFE = nc.declare_dram_parameter("zfe", [IN_NODE, W * slots],
                                    dt.bfloat16, isOutput=False)
    AQ = nc.declare_dram_parameter("aq", [IN_NODE, W * t_w, 2, 128],
                                   dt.float8e4, isOutput=False)
    OHM = nc.declare_dram_parameter("ohm", [128, W, t_w, 128],
                                    dt.float8e4, isOutput=False)
    WQ = nc.declare_dram_parameter("wq", [IN_NODE, 2, FE], dt.float8e4,
                                   isOutput=False)
    WPAY = nc.declare_dram_parameter("wpay", [IN_NODE, NPAY], dt.bfloat16,
                                     isOutput=False)
    ATTN = nc.declare_dram_parameter("attn_rep", [128, FE], dt.bfloat16,
                                     isOutput=False)
    BMEAN = nc.declare_dram_parameter("bmean", [128, OUT_NODE], dt.float32,
                                      isOutput=False)
    OUT = nc.declare_dram_parameter("out", [W * 128, OUT_NODE], dt.float32,
                                    isOutput=True)
    DBG = None
    if os.environ.get("EGAT_TAPS"):
        DBG = dict(
            lr=nc.declare_dram_parameter("dbg_lr", [128, W, t_w * FE],
                                         dt.bfloat16, isOutput=True),
            wgt=nc.declare_dram_parameter("dbg_wgt", [128, W, t_w * H],
                                          dt.bfloat16, isOutput=True),
        )

    with tile.TileContext(nc) as tc, ExitStack() as ctx:
        cpool = ctx.enter_context(tc.tile_pool(name="consts", bufs=1))
        wq_s = cpool.tile([128, 2, FE], dt.float8e4)
        nc.sync.dma_start(wq_s[:], WQ[:])
        wpay_s = cpool.tile([128, NPAY], dt.bfloat16)
        nc.sync.dma_start(wpay_s[:], WPAY[:])
        attn_s = cpool.tile([128, FE], dt.bfloat16)
        nc.sync.dma_start(attn_s[:], ATTN[:])
        bmean_s = cpool.tile([128, OUT_NODE], dt.float32)
        nc.sync.dma_start(bmean_s[:], BMEAN[:])
        out_acc = cpool.tile([128, W, OUT_NODE], dt.float32)

        OUT_CHUNK = 5

        with tc.tile_pool(name="feat", bufs=4) as fpool, \
             tc.tile_pool(name="rhsp", bufs=3) as rpool, \
             tc.tile_pool(name="work", bufs=3) as wpool, \
             tc.tile_pool(name="stgp", bufs=3) as spool, \
             tc.tile_pool(name="ep", bufs=3) as epool, \
             tc.tile_pool(name="psA", bufs=1, space="PSUM") as psA, \
             tc.tile_pool(name="psC", bufs=2, space="PSUM") as psC, \
             tc.tile_pool(name="psP", bufs=2, space="PSUM") as psP:
            out_done = [0]

            def flush_out(upto):
                lo = out_done[0]
                if upto > lo:
                    nc.sync.dma_start(
                        OUT[lo * 128:upto * 128, :].rearrange(
                            "(w p) c -> p w c", p=128),
                        out_acc[:, lo:upto, :])
                    out_done[0] = upto

            def finish_window(w, ohw, rhs):
                P = psP.tile([128, NR], dt.float32, tag="P")
                for t in range(t_w):
                    nc.tensor.matmul(P[:], lhsT=ohw[:, t, :],
                                     rhs=rhs[:, t, :],
                                     start=(t == 0), stop=(t == t_w - 1),
                                     skip_group_check=True)
                sg = epool.tile([128, H], dt.float32, tag="sg")
                nc.vector.tensor_scalar(out=sg[:], in0=P[:, NPAY:],
                                        scalar1=1e-30, scalar2=None,
                                        op0=OP.max)
                si = epool.tile([128, H], dt.float32, tag="si")
                nc.vector.reciprocal(si[:], sg[:])
                m01 = epool.tile([128, 1], dt.float32, tag="m01")
                nc.vector.tensor_scalar(out=m01[:], in0=P[:, NPAY:NPAY + 1],
                                        scalar1=0.0, scalar2=None,
                                        op0=OP.is_gt)
                tmp = epool.tile([128, OUT_NODE, H], dt.float32, tag="tmp")
                nc.vector.tensor_tensor(
                    out=tmp[:],
                    in0=P[:, 0:NPAY].rearrange("p (f h) -> p f h", h=H),
                    in1=si[:].unsqueeze(1).broadcast_to([128, OUT_NODE, H]),
                    op=OP.mult)
                # head sum (pairwise on gpsimd), + bmean, relu, mask
                t2 = epool.tile([128, OUT_NODE, 2], dt.float32, tag="t2")
                nc.gpsimd.tensor_tensor(out=t2[:], in0=tmp[:, :, 0:2],
                                        in1=tmp[:, :, 2:4], op=OP.add)
                acc = epool.tile([128, OUT_NODE], dt.float32, tag="acc")
                nc.gpsimd.tensor_tensor(out=acc[:].unsqueeze(2),
                                        in0=t2[:, :, 0:1], in1=t2[:, :, 1:2],
                                        op=OP.add)
                acc2 = epool.tile([128, OUT_NODE], dt.float32, tag="acc2")
                nc.gpsimd.tensor_tensor(out=acc2[:], in0=acc[:],
                                        in1=bmean_s[:], op=OP.add)
                nc.gpsimd.tensor_scalar(out=out_acc[:, w, :], in0=acc2[:],
                                        scalar1=0.0, scalar2=m01[:],
                                        op0=OP.max, op1=OP.mult)
                if w + 1 >= out_done[0] + OUT_CHUNK:
                    flush_out(w + 1)

            pend = None   # (w, ohw, rhs) awaiting scatter+epilogue
            for w in range(W):
                zfe = fpool.tile([128, slots], dt.bfloat16, tag="zfe")
                nc.sync.dma_start(zfe[:], ZFE[:, w * slots:(w + 1) * slots])
                aqw = fpool.tile([128, t_w, 2, 128], dt.float8e4, tag="aq")
                nc.sync.dma_start(aqw[:],
                                  AQ[:, w * t_w:(w + 1) * t_w, :, :])
                ohw = fpool.tile([128, t_w, 128], dt.float8e4, tag="ohw")
                nc.gpsimd.dma_start(ohw[:], OHM[:, w, :, :])

                rhs = rpool.tile([128, t_w, NR], dt.bfloat16, tag="rhs")
                lr_sb = wpool.tile([128, t_w, FE], dt.bfloat16, tag="lr")
                wgt = wpool.tile([128, t_w, H], dt.bfloat16, tag="wgt")

                A = psA.tile([128, t_w, FE], dt.float32, tag="A")
                for hf in range(2):
                    ts0 = hf * t_half
                    # ---- projections ------------------------------------
                    Cs = []
                    for q in range(2):
                        tq0 = ts0 + 4 * q
                        C = psC.tile([128, 4, NPAY], dt.float32, tag="C")
                        Cs.append(C)
                        for k in range(4):
                            t = tq0 + k
                            c0 = t * 128
                            sl = slice(c0, c0 + 128)
                            nc.tensor.matmul(C[:, k, :], lhsT=zfe[:, sl],
                                             rhs=wpay_s[:], start=True,
                                             stop=True, skip_group_check=True)
                            # fused src+dst attention projection: fp8
                            # DoubleRow pairs the two k-tiles (nfe, dfe)
                            nc.tensor.matmul(
                                A[:, t, :], lhsT=aqw[:, t, :, :],
                                rhs=wq_s[:], start=True, stop=True,
                                perf_mode=mybir.MatmulPerfMode.DoubleRow,
                                skip_group_check=True)

                    # ---- attention chain (half granularity) -------------
                    # lr = leaky_relu(A)  [ACT, PSUM->SBUF bf16]
                    # Prelu == leaky relu with alpha slope; unlike Lrelu it
                    # shares the exp_and_others table with Exp/Copy, so the
                    # ACT engine never reloads its function table mid-window
                    nc.scalar.activation(lr_sb[:, ts0:ts0 + t_half, :],
                                         A[:, ts0:ts0 + t_half, :],
                                         AF.Prelu, alpha=SLOPE)
                    # ea = lr * attn  [GPSIMD]
                    ea = wpool.tile([128, t_half, H, OUT_EDGE], dt.bfloat16,
                                    tag="ea")
                    ea_eng = nc.vector if DBG_EA_DVE else nc.gpsimd
                    ea_eng.tensor_tensor(
                        out=ea[:].rearrange("p t h f -> p t (h f)"),
                        in0=lr_sb[:, ts0:ts0 + t_half, :],
                        in1=attn_s[:].unsqueeze(1).broadcast_to(
                            [128, t_half, FE]),
                        op=OP.mult)
                    # eat = sum_f ea  [GPSIMD pairwise]
                    eat = wpool.tile([128, t_half, H], dt.float32, tag="eat")
                    if DBG_EAT_DVE:
                        nc.vector.tensor_reduce(
                            eat[:], ea[:], axis=mybir.AxisListType.X,
                            op=OP.add)
                    else:
                        e8 = wpool.tile([128, t_half, H, 8], dt.bfloat16,
                                        tag="e8")
                        nc.gpsimd.tensor_tensor(out=e8[:],
                                                in0=ea[:, :, :, 0:8],
                                                in1=ea[:, :, :, 8:16],
                                                op=OP.add)
                        e4 = wpool.tile([128, t_half, H, 4], dt.bfloat16,
                                        tag="e4")
                        nc.gpsimd.tensor_tensor(out=e4[:],
                                                in0=e8[:, :, :, 0:4],
                                                in1=e8[:, :, :, 4:8],
                                                op=OP.add)
                        e2 = wpool.tile([128, t_half, H, 2], dt.bfloat16,
                                        tag="e2")
                        nc.gpsimd.tensor_tensor(out=e2[:],
                                                in0=e4[:, :, :, 0:2],
                                                in1=e4[:, :, :, 2:4],
                                                op=OP.add)
                        nc.gpsimd.tensor_tensor(out=eat[:].unsqueeze(3),
                                                in0=e2[:, :, :, 0:1],
                                                in1=e2[:, :, :, 1:2],
                                                op=OP.add)
                    # wgt = exp(eat)  [ACT]
                    nc.scalar.activation(wgt[:, ts0:ts0 + t_half, :], eat[:],
                                         AF.Exp)
                    # denominator columns of rhs  [DVE, small]
                    nc.gpsimd.tensor_scalar(
                        out=rhs[:, ts0:ts0 + t_half, NPAY:],
                        in0=wgt[:, ts0:ts0 + t_half, :],
                        scalar1=1.0, scalar2=None, op0=OP.mult)

                    # ---- payload x wgt ----------------------------------
                    for q in range(2):
                        tq0 = ts0 + 4 * q
                        C = Cs[q]
                        wv = (wgt[:, tq0:tq0 + 4, :].unsqueeze(2)
                              .broadcast_to([128, 4, OUT_NODE, H]))
                        rv = rhs[:, tq0:tq0 + 4, 0:NPAY].rearrange(
                            "p t (f h) -> p t f h", h=H)
                        if q == 0:
                            # ACT stages PSUM->SBUF bf16; the weight multiply
                            # runs on DVE (half 0) / Pool (half 1) to balance
                            stg = spool.tile([128, 4, NPAY], dt.bfloat16,
                                             tag="stg")
                            nc.scalar.copy(stg[:], C[:])
                            meng = nc.vector if hf == 0 else nc.gpsimd
                            meng.tensor_tensor(
                                out=rv,
                                in0=stg[:].rearrange("p t (f h) -> p t f h",
                                                     h=H),
                                in1=wv, op=OP.mult)
                        else:
                            # DVE fused from PSUM (1x)
                            nc.vector.tensor_tensor(
                                out=rv,
                                in0=C[:].rearrange("p t (f h) -> p t f h",
                                                   h=H),
                                in1=wv, op=OP.mult)

                if DBG is not None:
                    nc.sync.dma_start(DBG["lr"][:, w, :],
                                      lr_sb[:].rearrange("p t f -> p (t f)"))
                    nc.sync.dma_start(DBG["wgt"][:, w, :],
                                      wgt[:].rearrange("p t h -> p (t h)"))
                # ---- scatter + epilogue of the PREVIOUS window ----------
                # (software pipelining: by the time window w-1's scatter
                # enters the PE queue, its rhs is long done, so the PE never
                # stalls on the attention chain)
                if pend is not None:
                    finish_window(*pend)
                pend = (w, ohw, rhs)
            if pend is not None:
                finish_window(*pend)
            flush_out(W)

    if not nc.is_finalized():
        nc.finalize()
    return nc


# ===========================================================================
# numpy emulation of the device program (for validation/debug)
# ===========================================================================

def emulate_core(in_map, W, cfg):
    t_half = cfg["t_half"]
    slots = 2 * t_half * 128
    f32 = np.float32

    wq = in_map["wq"].astype(f32)
    wpay = in_map["wpay"].astype(f32)
    attn_rep = in_map["attn_rep"][0].astype(f32)
    bmean = in_map["bmean"][0]
    t_w = 2 * t_half

    out = np.zeros((W * 128, OUT_NODE), f32)
    for w in range(W):
        nfe = in_map["zfe"][:, w * slots:(w + 1) * slots].astype(f32).T
        aq = in_map["aq"][:, w * t_w:(w + 1) * t_w].astype(f32)
        nfe8 = aq[:, :, 0, :].reshape(IN_NODE, slots).T
        dfe8 = aq[:, :, 1, :].reshape(IN_NODE, slots).T
        A = nfe8 @ wq[:, 0, :] + dfe8 @ wq[:, 1, :]   # [slots, 64] psum f32
        Cp = nfe @ wpay                               # [slots, 256] (f,h)
        lr = np.maximum(A, SLOPE * A).astype(BF16).astype(f32)
        ea = (lr * attn_rep[None, :]).astype(BF16)
        ea = ea.reshape(-1, H, OUT_EDGE)
        e8 = (ea[:, :, 0:8] + ea[:, :, 8:16]).astype(BF16)
        e4 = (e8[:, :, 0:4] + e8[:, :, 4:8]).astype(BF16)
        e2 = (e4[:, :, 0:2] + e4[:, :, 2:4]).astype(BF16)
        eat = (e2[:, :, 0].astype(f32) + e2[:, :, 1].astype(f32))
        wgt = np.exp(eat).astype(BF16).astype(f32)    # [slots, H]
        stg = Cp.astype(BF16).astype(f32)
        rhs_pay = (stg.reshape(-1, OUT_NODE, H)
                   * wgt[:, None, :]).astype(BF16).astype(f32)
        rhs = np.concatenate([rhs_pay.reshape(-1, NPAY), wgt], axis=1)
        oh = (in_map["ohm"][:, w].astype(f32).transpose(1, 0, 2)
              .reshape(slots, 128))
        P = oh.T @ rhs.astype(BF16).astype(f32)       # [128, 260]
        s = np.maximum(P[:, NPAY:], 1e-30)
        si = 1.0 / s
        tmp = P[:, 0:NPAY].reshape(128, OUT_NODE, H) * si[:, None, :]
        acc = tmp.sum(axis=2)
        m01 = (P[:, NPAY:NPAY + 1] > 0).astype(f32)
        out[w * 128:(w + 1) * 128] = np.maximum(acc + bmean[None, :], 0) * m01
    return out


def assemble(meta, results):
    n_dst = meta["cfg"]["n_dst"]
    out = np.zeros((n_dst, OUT_NODE), np.float32)
    for c in range(N_CORES):
        slots_rows, glob_rows = meta["asm"][c]
        if len(glob_rows):
            out[glob_rows] = results[c]["out"][slots_rows]
    return out


# ===========================================================================
# entry point
# ===========================================================================

_CACHE = {}
LAST_EXEC_NS = None
LAST_RESULT = None


def kernel(nfeats, dst_feats, reward, src, dst,
           W_ns, b_ns, W_ni, W_nj, W_fij, attn, b_e):
    global LAST_EXEC_NS, LAST_RESULT
    from concourse.bass_utils import run_bass_kernel_spmd

    meta, in_maps = prep(nfeats, dst_feats, reward, src, dst,
                         W_ns, b_ns, W_ni, W_nj, W_fij, attn, b_e)
    key = meta["W"]
    if key not in _CACHE:
        _CACHE[key] = build_program(meta["W"], meta["cfg"])
    nc = _CACHE[key]
    kwargs = {}
    if os.environ.get("EGAT_TRACE"):
        kwargs = dict(trace=True)
    try:
        res = run_bass_kernel_spmd(nc, in_maps, list(range(N_CORES)), **kwargs)
    except ModuleNotFoundError:
        res = run_bass_kernel_spmd(nc, in_maps, list(range(N_CORES)))
    LAST_EXEC_NS = res.exec_time_ns
    LAST_RESULT = res
    return assemble(meta, res.results)


def estimate_ns(W=None, cfg=None):
    """Cost-model (no_exec CoreSim) estimate of the per-core kernel time."""
    from concourse.bass_interp import CoreSim
    cfg = cfg or default_cfg()
    if W is None:
        W = sorted(_CACHE)[0] if _CACHE else 50
    nc = _CACHE.get(W) or build_program(W, cfg)
    sim = CoreSim(nc, no_exec=True)
    sim.simulate()
    return int(sim.time)
